# revision 30
# baseline (speedup 1.0000x reference)
"""Trainium2 Bass kernel for nn_Attention_10282151707309.

Reference computation:
  - channel LayerNorm over C=128 (biased var, eps=1e-5, affine g/b)
  - qkv = w_qkv @ xn (1x1 conv), 4 heads x 32 dims, q scaled by 1/sqrt(32)
  - full softmax attention over HW=4096 positions per (batch, head)
  - out = w_out @ attn_out + b_out

Sharding: 8 cores = (batch b in 0..3) x (spatial half in 0..1), SPMD
identical program; per-core x is the batch slice spatially rolled so the
core's own 2048 query columns are program-columns 0:2048 (attention is
permutation-equivariant over key positions).

Kernel design (per core):
  - LN via centered moments: mean broadcast by an all-(1/C) f32r matmul
    (every output row = column mean), xc = x - mean_bc (bf16), var row
    from a (1/C)-ones matmul of xc^2, rstd = exp(-0.5 ln(var+eps)) on
    ACT (same table set as softmax exp), broadcast back via a K=1 ones
    matmul, xn = xc * rstd_bc.
  - sims: per (it, jc, pair) two row-tiled K=32 bf16 matmuls (heads p and
    p+2 at tile_position rows 32p/32(p+2)) into one [128,1024] PSUM duo
    (4x less PE time than zero-padded K=128 matmuls).
  - exp split across engines: ACT does cols [0:XSPL) (true exp, bf16
    out), DVE does the rest via a Schraudolph bit-trick: round(A*s + B)
    written as int16 into a bf16-bitcast view (~+-3.3% per-element, the
    num/den ratio cancels most of it; validated ~1.4e-2 end-to-end).
  - av: col-tiled M=64 matmuls, lhsT = [ones(32) | v^T(32)] per (jc,
    head), so PSUM pair rows 0:32/64:96 accumulate the softmax
    denominator REPLICATED on 32 partitions and rows 32:64/96:128 the
    head dims. No broadcast needed at normalize time: rec = exp(-ln(den))
    on ACT stays on the replica partitions, and the cat multiply is a
    mixed-base DVE TT (PSUM dims rows x SBUF rec rows).
  - y = wo_a @ catA + wo_b @ catB accumulated straight back into the
    pair bank (freed by the cat reads), bias-add on the ACT copy out.
"""

import numpy as np

HEADS = 4
DIM_HEAD = 32
B, C, H, W = 4, 128, 64, 64
S = H * W              # 4096 spatial positions
HALF = S // 2          # 2048 own query columns per core
TI = 512               # query tile
NIT = HALF // TI       # 4 query tiles
JCHUNK = 128           # key chunk per step
NJC = S // JCHUNK      # 32 key chunks
EPS = 1e-5
N_CORES = 8

XSPL = 512             # exp columns done by ACT per 1024-col duo; rest DVE
                       # (bank-aligned: ACT and DVE never touch the same
                       # PSUM bank, so their exp halves run concurrently)
SCHRA = 128.0 / float(np.log(2.0))   # 184.6617
SCHRB = 16256.0 - 5.5                # bf16 exponent bias + centering

_PROGRAM = None


def _build_program():
    import concourse.bass as bass  # noqa: F401
    import concourse.mybir as mybir
    import concourse.tile as tile
    from concourse import bacc
    from concourse.bass import ts

    dt = mybir.dt.float32
    dtr = mybir.dt.float32r
    dtb = mybir.dt.bfloat16
    i16 = mybir.dt.int16
    F = mybir.ActivationFunctionType
    Op = mybir.AluOpType

    class PinnedTableBacc(bacc.Bacc):
        """Bacc whose activation-table chooser is pinned to the one set that
        holds every function this kernel uses (ln, exp, identity), so the
        whole kernel needs exactly one ACT_TABLE_LOAD instead of ping-ponging
        between the `natural_log` and `exp_and_others` sets (~1.3us each).
        Table list order/positions are preserved so set ids stay valid."""

        def insert_act_table_loads(self):
            import bass_rust as _bass_rust
            from concourse.hw_specs import get_activation_tables

            has_activation = any(
                isinstance(i, mybir.InstActivation)
                for b in self.main_func.blocks
                for i in b.instructions
            )
            if not has_activation:
                return
            pinned = {F.Ln, F.Exp, F.Identity, F.Copy}
            tables = [
                (name, fns if name == "natural_log_exp_and_others" else fns - pinned)
                for name, fns in get_activation_tables(self.m.arch).items()
            ]
            _bass_rust.insert_act_table_loads(self, tables)

    nc = PinnedTableBacc(
        "TRN2",
        target_bir_lowering=False,
        debug=False,
        num_devices=N_CORES,
    )

    x_d = nc.dram_tensor("x", [C, S], dt, kind="ExternalInput").ap()
    wq_d = nc.dram_tensor("wq_t", [C, 128], dtb, kind="ExternalInput").ap()
    wk_d = nc.dram_tensor("wk_t", [C, 128], dtb, kind="ExternalInput").ap()
    wv_d = nc.dram_tensor("wv_t", [C, 128], dtb, kind="ExternalInput").ap()
    woa_d = nc.dram_tensor("wo_a", [128, 128], dtb, kind="ExternalInput").ap()
    wob_d = nc.dram_tensor("wo_b", [128, 128], dtb, kind="ExternalInput").ap()
    bo_d = nc.dram_tensor("bias_o", [128, 1], dt, kind="ExternalInput").ap()
    y_d = nc.dram_tensor("y", [C, HALF], dt, kind="ExternalOutput").ap()

    with tile.TileContext(nc) as tc:
        from contextlib import ExitStack

        with ExitStack() as ctx:
            const_pool = ctx.enter_context(tc.tile_pool(name="const", bufs=1))
            big_pool = ctx.enter_context(tc.tile_pool(name="big", bufs=1))

            wq = const_pool.tile([C, 128], dtb, tag="wq")
            wk = const_pool.tile([C, 128], dtb, tag="wk")
            wv = const_pool.tile([C, 128], dtb, tag="wv")
            woa = const_pool.tile([128, 128], dtb, tag="woa")
            wob = const_pool.tile([128, 128], dtb, tag="wob")
            bo = const_pool.tile([128, 1], dt, tag="bo")
            nc.sync.dma_start(wq[:], wq_d[:])
            nc.sync.dma_start(wk[:], wk_d[:])
            nc.sync.dma_start(wv[:], wv_d[:])
            nc.gpsimd.dma_start(woa[:], woa_d[:])
            nc.gpsimd.dma_start(wob[:], wob_d[:])
            nc.gpsimd.dma_start(bo[:], bo_d[:])
            epsc = const_pool.tile([128, 1], dt, tag="epsc")
            nc.vector.memset(epsc[:], EPS)
            # all-(1/C) fp32 lhsT: mean broadcast matmul
            onesC = const_pool.tile([128, 128], dt, tag="onesC")
            nc.vector.memset(onesC[:], 1.0 / C)
            # [128,1] 1/C bf16 lhsT for the variance row matmul
            onesCb = const_pool.tile([128, 1], dtb, tag="onesCb")
            nc.vector.memset(onesCb[:], 1.0 / C)
            # ones bf16 lhsT rows (partitions 0 and 32) for the rstd
            # broadcast matmuls of even/odd LN tiles
            onesb = const_pool.tile([33, 128], dtb, tag="onesb")
            nc.vector.memset(onesb[:], 1.0)

            q_sb = big_pool.tile([128, HALF], dtb, tag="q")
            k_sb = big_pool.tile([128, S], dtb, tag="k")
            # vaug: per (jc, head) a [128, 64] av lhsT block:
            # cols 0:32 = ones (denominator replicas), cols 32:64 = v^T dims
            vaug = big_pool.tile([128, NJC * HEADS * 64], dtb, tag="vaug")
            ones_half = vaug[:].rearrange("p (b x) -> p b x", x=64)[:, :, 0:32]
            nc.gpsimd.memset(ones_half, 1.0)

            # ---------------- LayerNorm + projections ----------------
            with (
                tc.tile_pool(name="ln_sb", bufs=3) as lnsb,
                tc.tile_pool(name="x_sb", bufs=3) as xsb,
                tc.tile_pool(name="mean_ps", bufs=2, space="PSUM") as meanps,
                tc.tile_pool(name="var_ps", bufs=2, space="PSUM") as varps,
                tc.tile_pool(name="rbc_ps", bufs=1, space="PSUM") as rbcps,
                tc.tile_pool(name="proj_ps", bufs=3, space="PSUM") as projps,
            ):
                # two-stage software pipeline: stage1(t+1) is emitted before
                # stage2(t) so each engine's FIFO interleaves work of two
                # tiles (e.g. DVE does xc(t+1)/xcsq(t+1) while ACT runs the
                # Ln/Exp of tile t) instead of serializing the per-tile
                # DVE->ACT->PE dependency chain.
                # var rows of tile pairs share one PSUM tile (rows 0 / 32)
                # so one Ln and one Exp serve two tiles (ACT cost is
                # free-size only, the partition count is free)
                var_of = {}

                def stage1(t):
                    sl = ts(t, 512)
                    xt = xsb.tile([C, 512], dt, tag="xt", name="xt")
                    nc.sync.dma_start(xt[:], x_d[:, sl])
                    mean_bc = meanps.tile([128, 512], dt, tag="mean", name="mean")
                    nc.tensor.matmul(mean_bc[:], onesC[:], xt[:])
                    xc = lnsb.tile([C, 512], dtb, tag="xc", name="xc")
                    nc.vector.tensor_tensor(xc[:], xt[:], mean_bc[:], Op.subtract)
                    xcsq = lnsb.tile([C, 512], dtb, tag="xcsq", name="xcsq")
                    nc.vector.tensor_tensor(xcsq[:], xc[:], xc[:], Op.mult)
                    if t % 2 == 0:
                        var_of[t // 2] = varps.tile(
                            [33, 512], dt, tag="var", name="var"
                        )
                    var = var_of[t // 2]
                    nc.tensor.matmul(
                        var[32 * (t % 2) : 32 * (t % 2) + 1, :],
                        onesCb[:],
                        xcsq[:],
                        tile_position=(0, 32 * (t % 2)),
                    )
                    return xc, t

                rstd_of = {}

                def stage2(t, xc, _t):
                    sl = ts(t, 512)
                    if t % 2 == 0:
                        # Ln/Exp over both rows {0, 32} at once; row 32 is
                        # tile t+1's var (its matmul is already emitted by
                        # the stage skew when this runs)
                        var = var_of[t // 2]
                        lnv = lnsb.tile([33, 512], dt, tag="lnv", name="lnv")
                        nc.scalar.activation(
                            lnv[0:33, :], var[0:33, :], F.Ln,
                            bias=epsc[0:33, 0:1],
                        )
                        rstd_of[t // 2] = lnsb.tile(
                            [33, 512], dtb, tag="rstd", name="rstd"
                        )
                        nc.scalar.activation(
                            rstd_of[t // 2][0:33, :], lnv[0:33, :], F.Exp,
                            scale=-0.5,
                        )
                    rstd = rstd_of[t // 2][32 * (t % 2) : 32 * (t % 2) + 1, :]
                    rbc = rbcps.tile([128, 512], dt, tag="rbc", name="rbc")
                    nc.tensor.matmul(
                        rbc[:],
                        onesb[32 * (t % 2) : 32 * (t % 2) + 1, :],
                        rstd,
                        tile_position=(32 * (t % 2), 0),
                    )
                    xn = lnsb.tile([C, 512], dtb, tag="xn", name="xn")
                    nc.vector.tensor_tensor(xn[:], xc[:], rbc[:], Op.mult)

                    if t < NIT:
                        qp = projps.tile([128, 512], dt, tag="proj", name="qp")
                        nc.tensor.matmul(qp[:], wq[:], xn[:])
                        nc.scalar.activation(q_sb[:, sl], qp[:], F.Identity)
                    kp = projps.tile([128, 512], dt, tag="proj", name="kp")
                    nc.tensor.matmul(kp[:], wk[:], xn[:])
                    nc.scalar.activation(k_sb[:, sl], kp[:], F.Identity)
                    vp = projps.tile([128, 512], dt, tag="proj", name="vp")
                    for cch in range(4):
                        nc.tensor.matmul(
                            vp[:, ts(cch, 128)], xn[:, ts(cch, 128)], wv[:]
                        )
                    # scatter v dims into the vaug blocks of this s-tile
                    vdst = (
                        vaug[:, t * 4 * HEADS * 64 : (t + 1) * 4 * HEADS * 64]
                        .rearrange("p (c h x) -> p c h x", c=4, x=64)[:, :, :, 32:64]
                    )
                    vsrc = vp[:].rearrange("p (c h d) -> p c h d", c=4, d=32)
                    nc.vector.tensor_copy(vdst, vsrc)

                carry = stage1(0)
                for t in range(S // 512):
                    nxt = stage1(t + 1) if t + 1 < S // 512 else None
                    stage2(t, *carry)
                    carry = nxt

            # ---------------- attention ----------------
            with (
                tc.tile_pool(name="duo_ps", bufs=3, space="PSUM") as duops,
                tc.tile_pool(name="pair_ps", bufs=1, space="PSUM") as pairps,
                tc.tile_pool(name="exp_sb", bufs=3) as expsb,
                tc.tile_pool(name="tail_sb", bufs=1) as tailsb,
                tc.tile_pool(name="y_sb", bufs=2) as ysbp,
            ):
                # persistent tail tiles (single-buffered; Tile serializes reuse)
                lns = tailsb.tile([128, 1024], dt, tag="lns")
                rec = tailsb.tile([128, 1024], dtb, tag="rec")
                cat = tailsb.tile([128, 1024], dtb, tag="cat")
                nc.gpsimd.memset(cat[:], 0.0)

                def emit_sims(it, jc, p):
                    isl = ts(it, TI)
                    jsl = ts(jc, JCHUNK)
                    duo = duops.tile([128, 1024], dt, tag="duo", name="duo")
                    ha, hb = p, p + 2
                    nc.tensor.matmul(
                        duo[:, 0:TI],
                        k_sb[32 * ha : 32 * ha + 32, jsl],
                        q_sb[32 * ha : 32 * ha + 32, isl],
                        tile_position=(32 * ha, 0),
                    )
                    nc.tensor.matmul(
                        duo[:, TI : 2 * TI],
                        k_sb[32 * hb : 32 * hb + 32, jsl],
                        q_sb[32 * hb : 32 * hb + 32, isl],
                        tile_position=(32 * hb, 0),
                    )
                    return duo

                def emit_burst(n):
                    # gapless junk matmuls: dependency-free back-to-back PE
                    # work that re-arms the HAM clock gate (K=8/8) across the
                    # pipeline-drain stalls. Results are never read.
                    j = duops.tile([128, 1024], dt, tag="duo", name="junk")
                    for _ in range(n):
                        nc.tensor.matmul(
                            j[:, 0:TI], wq[:], k_sb[:, 0:TI],
                            skip_group_check=True,
                        )

                def emit_tail(it, pairA, pairB):
                    # pair rows 0:32 / 64:96 hold denominator replicas,
                    # rows 32:64 / 96:128 the head dims. Per-pair chains so
                    # pairA's bank is released (for the next it's avs) after
                    # just its own Ln/Exp/cat ops. Ln of the dim rows gives
                    # unused NaNs. y accumulates into a junk duo slot.
                    nc.scalar.activation(lns[0:96, 0:TI], pairA[0:96, :], F.Ln)
                    nc.scalar.activation(
                        rec[0:96, 0:TI], lns[0:96, 0:TI], F.Exp, scale=-1.0
                    )
                    nc.vector.tensor_tensor(
                        cat[32:64, 0:TI], pairA[32:64, :], rec[0:32, 0:TI], Op.mult
                    )
                    nc.vector.tensor_tensor(
                        cat[96:128, 0:TI], pairA[96:128, :], rec[64:96, 0:TI], Op.mult
                    )
                    nc.scalar.activation(lns[0:96, TI:1024], pairB[0:96, :], F.Ln)
                    nc.scalar.activation(
                        rec[0:96, TI:1024], lns[0:96, TI:1024], F.Exp, scale=-1.0
                    )
                    nc.vector.tensor_tensor(
                        cat[32:64, TI:1024], pairB[32:64, :], rec[0:32, TI:1024],
                        Op.mult,
                    )
                    nc.vector.tensor_tensor(
                        cat[96:128, TI:1024], pairB[96:128, :], rec[64:96, TI:1024],
                        Op.mult,
                    )
                    emit_burst(14 if it < NIT - 1 else 6)
                    yp = duops.tile([128, 1024], dt, tag="duo", name="yp")
                    nc.tensor.matmul(
                        yp[:, 0:TI], woa[:], cat[:, 0:TI],
                        start=True, stop=False, skip_group_check=True,
                    )
                    nc.tensor.matmul(
                        yp[:, 0:TI], wob[:], cat[:, TI : 2 * TI],
                        start=False, stop=True, skip_group_check=True,
                    )
                    ysb = ysbp.tile([128, TI], dt, tag="ysb")
                    nc.scalar.activation(
                        ysb[:], yp[:, 0:TI], F.Identity, bias=bo[:, 0:1]
                    )
                    nc.sync.dma_start(y_d[:, ts(it, TI)], ysb[:])

                steps = [
                    (it, jc, p)
                    for it in range(NIT)
                    for jc in range(NJC)
                    for p in range(2)
                ]

                emit_burst(10)

                pairs_of_it = {}
                # 2-step sim lead: the avs of step s sit in the PE FIFO
                # behind the sims of steps s+1 AND s+2, so the ~1us exp
                # latency of step s is fully hidden by PE work
                duoq = [emit_sims(*steps[0]), emit_sims(*steps[1])]
                for si, (it, jc, p) in enumerate(steps):
                    if (jc, p) == (0, 0):
                        pairs_of_it[it] = (
                            pairps.tile([128, TI], dt, tag="pairA", name="pairA"),
                            pairps.tile([128, TI], dt, tag="pairB", name="pairB"),
                        )
                    pair = pairs_of_it[it][p]
                    duo = duoq.pop(0)
                    # whole-duo exp ops, engine alternating by step parity
                    # (parity == p, so each (head, query) softmax row is
                    # consistently exact-exp or Schraudolph-exp): one big op
                    # amortizes the fixed per-op cost, and ACT/DVE never
                    # touch the same PSUM bank.
                    if p == 0:
                        et = expsb.tile([128, 1024], dtb, tag="eta", name="eta")
                        nc.scalar.activation(et[:], duo[:], F.Exp)
                        eta, etb = et[:, 0:TI], et[:, TI : 2 * TI]
                    else:
                        et = expsb.tile([128, 1024], i16, tag="etb", name="etb")
                        nc.vector.tensor_scalar(
                            et[:], duo[:], SCHRA, SCHRB, Op.mult, Op.add
                        )
                        eta = et[:, 0:TI].bitcast(dtb)
                        etb = et[:, TI : 2 * TI].bitcast(dtb)
                    if si + 2 < len(steps):
                        duoq.append(emit_sims(*steps[si + 2]))
                    ha, hb = p, p + 2
                    st, sp_ = jc == 0, jc == NJC - 1
                    nc.tensor.matmul(
                        pair[0:64, :],
                        vaug[:, (jc * HEADS + ha) * 64 : (jc * HEADS + ha) * 64 + 64],
                        eta,
                        tile_position=(0, 0),
                        start=st,
                        stop=sp_,
                        skip_group_check=True,
                    )
                    nc.tensor.matmul(
                        pair[64:128, :],
                        vaug[:, (jc * HEADS + hb) * 64 : (jc * HEADS + hb) * 64 + 64],
                        etb,
                        tile_position=(0, 64),
                        start=st,
                        stop=sp_,
                        skip_group_check=True,
                    )

                    if (jc, p) == (NJC - 1, 1):
                        emit_tail(it, *pairs_of_it[it])

    nc.compile()
    return nc


def _get_program():
    global _PROGRAM
    if _PROGRAM is None:
        _PROGRAM = _build_program()
    return _PROGRAM


def _prep_inputs(x, g, b, w_qkv, w_out, b_out):
    """Host-side sharding + weight folding. All tiny except x slicing."""
    f32 = np.float32
    x = np.asarray(x, f32).reshape(B, C, S)
    g_ = np.asarray(g, f32).reshape(C)
    b_ = np.asarray(b, f32).reshape(C)
    w_qkv = np.asarray(w_qkv, f32)
    w_out = np.asarray(w_out, f32)
    b_out = np.asarray(b_out, f32)

    import ml_dtypes

    bf16 = ml_dtypes.bfloat16
    scale = DIM_HEAD ** -0.5
    wg = w_qkv * g_[None, :]
    hid = HEADS * DIM_HEAD  # 128
    wq_t = np.ascontiguousarray((wg[0:hid] * scale).T).astype(bf16)
    wk_t = np.ascontiguousarray(wg[hid : 2 * hid].T).astype(bf16)
    wv_t = np.ascontiguousarray(wg[2 * hid : 3 * hid].T).astype(bf16)

    # v-bias folds exactly into the output bias (softmax rows sum to 1)
    bias_qkv = w_qkv @ b_
    bias_v = bias_qkv[2 * hid : 3 * hid]
    bias_o = np.ascontiguousarray((b_out + w_out @ bias_v).reshape(128, 1)).astype(f32)

    wo_t = w_out.T  # [hd, o]
    wo_a = np.zeros((128, 128), f32)
    wo_b = np.zeros((128, 128), f32)
    wo_a[32:64] = wo_t[0:32]      # head 0 dims sit at cat rows 32:64
    wo_a[96:128] = wo_t[64:96]    # head 2 dims at cat rows 96:128
    wo_b[32:64] = wo_t[32:64]     # head 1
    wo_b[96:128] = wo_t[96:128]   # head 3
    wo_a = wo_a.astype(bf16)
    wo_b = wo_b.astype(bf16)

    shared = {
        "wq_t": wq_t,
        "wk_t": wk_t,
        "wv_t": wv_t,
        "wo_a": wo_a,
        "wo_b": wo_b,
        "bias_o": bias_o,
    }
    in_maps = []
    for core in range(N_CORES):
        bb, half = core // 2, core % 2
        if half == 0:
            xc = x[bb]
        else:
            xc = np.concatenate([x[bb][:, HALF:], x[bb][:, :HALF]], axis=1)
        m = {"x": np.ascontiguousarray(xc)}
        m.update(shared)
        in_maps.append(m)
    return in_maps


def _run(inputs, trace=False):
    from concourse.bass_utils import run_bass_kernel_spmd

    nc = _get_program()
    in_maps = _prep_inputs(**inputs)
    res = run_bass_kernel_spmd(
        nc, in_maps, core_ids=list(range(N_CORES)), trace=trace
    )
    y = np.empty((B, C, S), np.float32)
    for core in range(N_CORES):
        bb, half = core // 2, core % 2
        yc = res.results[core]["y"]
        if half == 0:
            y[bb][:, :HALF] = yc
        else:
            y[bb][:, HALF:] = yc
    return y.reshape(B, C, H, W), res


def kernel(x, g, b, w_qkv, w_out, b_out):
    out, _ = _run(
        {"x": x, "g": g, "b": b, "w_qkv": w_qkv, "w_out": w_out, "b_out": b_out}
    )
    return out
